# revision 13
# baseline (speedup 1.0000x reference)
"""APPNP layer (GNN message passing) on 8 TRN2 NeuronCores.

Algorithm (reference):
    support = x @ W                         # [N, 96]
    h = support
    repeat 10x:  h = relu(0.9 * SpMM(A, h) + 0.1 * support)

Distribution: dst-shard nodes across 8 cores (6250 each). Each iteration:
  - every core keeps a full bf16 replica of h (rows padded to 128 cols) in
    DRAM, refreshed by AllGather of the 8 shard updates;
  - SpMM: edges sorted by dst, per-node runs padded to multiples of 4 and
    packed into 128-edge tiles; h[src] rows fetched with gpsimd.dma_gather
    (int16 indices -> the replica is split into lo/hi halves of 25088 rows,
    each node's edges split into a lo-run and a hi-run);
  - scale by 0.9*val on DVE (broadcast along features);
  - segment-sum via a single constant stationary matmul per 4 tiles:
    lhsT = LD4[k,m] = (k//4 <= m) gives prefix sums at 4-edge ("virtual")
    granularity -> PSUM [32, 512];
  - virtual prefixes staged to DRAM; per-node (start,end) boundary rows
    dma_gathered back; two diffs (lo+hi) + 0.1*support, relu, cast bf16;
  - AllGather shard -> next iteration's replica.

kernel(**inputs) accepts FULL inputs and returns the FULL [50000, 96] output.
"""

import numpy as np

_DEF = dict(N=50000, E=800000, IN_F=512, OUT_F=96, ALPHA=0.1, ITERS=10, CORES=8)

OUT_F = 96
FW = 128          # padded feature width (256B bf16 / 512B f32 rows)
P = 128
VPT = 32          # virtuals (4-edge groups) per 128-edge tile
MM_TILES = 4      # tiles per segment matmul (N = 4*128 = 512)
CH_TILES = 32     # tiles per gather chunk (4096 indices/call)


def _wrap16(idx):
    """[n] int -> dma_gather idx layout [128, n//16] int16."""
    n = idx.shape[0]
    assert n % 16 == 0
    return np.tile(idx.reshape(n // 16, 16).T, (8, 1)).astype(np.int16)


# ----------------------------------------------------------------------------
# Host-side preprocessing
# ----------------------------------------------------------------------------

def _prep(inputs, cfg):
    N, IN_F, C = cfg["N"], cfg["IN_F"], cfg["CORES"]
    ALPHA = cfg["ALPHA"]
    x = np.asarray(inputs["x"], np.float32)
    w = np.asarray(inputs["weight"], np.float32)
    src = np.asarray(inputs["edge_src"], np.int64)
    dst = np.asarray(inputs["edge_dst"], np.int64)
    val = np.asarray(inputs["edge_val"], np.float32)

    SHARD = N // C
    NB = -(-SHARD // P)
    SP = NB * P
    HALF = (C // 2) * SP          # row count of each half-table

    def grow(s):
        owner = s // SHARD
        local = s % SHARD
        return owner * SP + (local % P) * NB + local // P

    gsrc = grow(src)
    owner = dst // SHARD
    local = dst % SHARD

    # pack per core, lo table then hi table
    packs = []
    for c in range(C):
        m = owner == c
        ldst, lsrc, lval = local[m], gsrc[m], val[m]
        order = np.argsort(ldst, kind="stable")
        ldst, lsrc, lval = ldst[order], lsrc[order], lval[order]
        counts = np.bincount(ldst, minlength=SHARD)
        starts = np.concatenate([[0], np.cumsum(counts)])

        halves = []
        for half in (0, 1):
            node_tv = np.full((SP, 3), -1, np.int64)   # tile, a, b
            items = []                                  # (d4, n, rows, vv)
            for n in range(SHARD):
                s0, s1 = starts[n], starts[n + 1]
                rows = lsrc[s0:s1]
                vv = lval[s0:s1]
                sel = (rows >= HALF) == bool(half)
                rows, vv = rows[sel] - half * HALF, vv[sel]
                d = rows.shape[0]
                if d == 0:
                    continue
                items.append((-(-d // 4) * 4, n, rows, vv))
            items.sort(key=lambda x: -x[0])
            fills, tiles_idx, tiles_val = [], [], []
            for d4, n, rows, vv in items:
                # first-fit decreasing (search recent bins first for speed)
                ti = -1
                for j in range(len(fills) - 1, max(-1, len(fills) - 64), -1):
                    if fills[j] + d4 <= P:
                        ti = j
                        break
                if ti < 0:
                    fills.append(0)
                    tiles_idx.append(np.zeros(P, np.int64))
                    tiles_val.append(np.zeros(P, np.float32))
                    ti = len(fills) - 1
                fill = fills[ti]
                tiles_idx[ti][fill:fill + rows.shape[0]] = rows
                tiles_val[ti][fill:fill + rows.shape[0]] = vv
                node_tv[n] = (ti, fill // 4, (fill + d4) // 4)
                fills[ti] = fill + d4
            if not tiles_idx:
                tiles_idx.append(np.zeros(P, np.int64))
                tiles_val.append(np.zeros(P, np.float32))
            halves.append((tiles_idx, tiles_val, node_tv))
        packs.append(halves)

    # global tile counts (same for every core; lo chunk-aligned)
    TL = max(len(p[0][0]) for p in packs)
    TL = -(-TL // CH_TILES) * CH_TILES
    TH = max(len(p[1][0]) for p in packs)
    TH = -(-TH // CH_TILES) * CH_TILES
    T = TL + TH
    assert 32 * (max(TL, TH) + 1) < 32768

    wp = np.zeros((P, (IN_F // P) * OUT_F), np.float32)
    for k in range(IN_F // P):
        wp[:, k * OUT_F:(k + 1) * OUT_F] = w[k * P:(k + 1) * P, :]
    ld4 = (np.arange(P)[:, None] // 4 <= np.arange(VPT)[None, :]).astype(np.float32)

    import ml_dtypes
    bf16 = ml_dtypes.bfloat16

    in_maps = []
    for c in range(C):
        idx_cols, val_arr = [], np.zeros((T, P), np.float32)
        bounds = []
        for half, base_t, tcap in ((0, 0, TL), (1, TL, TH)):
            tiles_idx, tiles_val, node_tv = packs[c][half]
            ti = np.zeros((tcap, P), np.int64)
            tv = np.zeros((tcap, P), np.float32)
            ti[:len(tiles_idx)] = np.stack(tiles_idx)
            tv[:len(tiles_val)] = np.stack(tiles_val)
            val_arr[base_t:base_t + tcap] = tv
            idx_cols.append(ti)
            be = np.full(SP, tcap, np.int64)          # zero row: v=0, t=tcap
            bs = np.full(SP, tcap, np.int64)
            tt, a, b = node_tv[:, 0], node_tv[:, 1], node_tv[:, 2]
            has = tt >= 0
            be[has] = (b[has] - 1) * (tcap + 1) + tt[has]
            m2 = has & (a > 0)
            bs[m2] = (a[m2] - 1) * (tcap + 1) + tt[m2]
            bounds += [be, bs]

        gidx = np.concatenate(idx_cols, 0)          # [T, 128] table-local rows
        idxg = np.concatenate(
            [_wrap16(gidx[ci * CH_TILES:(ci + 1) * CH_TILES].ravel())
             for ci in range(T // CH_TILES)], axis=1)

        lo, hi = c * SHARD, (c + 1) * SHARD
        xt = np.zeros((IN_F, SP), np.float32)
        xt[:, :SHARD] = x[lo:hi].T

        in_maps.append(dict(
            xt=xt, wp=wp, ld4=ld4.astype(bf16),
            idxg=idxg,
            vals=np.ascontiguousarray((val_arr * (1 - ALPHA)).T).astype(bf16),
            bel=_wrap16(bounds[0]), bsl=_wrap16(bounds[1]),
            beh=_wrap16(bounds[2]), bsh=_wrap16(bounds[3]),
        ))

    meta = dict(T=T, TL=TL, NB=NB, SP=SP, SHARD=SHARD, HALF=HALF)
    return in_maps, meta


# ----------------------------------------------------------------------------
# Device kernel
# ----------------------------------------------------------------------------

def _build(cfg, T, TL, NB, debug=False):
    import concourse.bacc as bacc
    import concourse.tile as tile
    from concourse import bass, mybir
    from concourse.library_config import mlp

    IN_F, ITERS, C = cfg["IN_F"], cfg["ITERS"], cfg["CORES"]
    ALPHA = cfg["ALPHA"]
    SP = NB * P
    HALF = (C // 2) * SP
    KC = IN_F // P
    F = OUT_F
    n_ch = T // CH_TILES

    nc = bacc.Bacc("TRN2", target_bir_lowering=False, debug=debug, num_devices=C)

    xt_d = nc.dram_tensor("xt", [IN_F, SP], mybir.dt.float32, kind="ExternalInput")
    wp_d = nc.dram_tensor("wp", [P, KC * F], mybir.dt.float32, kind="ExternalInput")
    ld4_d = nc.dram_tensor("ld4", [P, VPT], mybir.dt.bfloat16, kind="ExternalInput")
    CW = CH_TILES * P // 16
    idxg_d = nc.dram_tensor("idxg", [P, n_ch * CW], mybir.dt.int16, kind="ExternalInput")
    vals_d = nc.dram_tensor("vals", [P, T], mybir.dt.bfloat16, kind="ExternalInput")
    bnd_d = {}
    for nm in ("bel", "bsl", "beh", "bsh"):
        bnd_d[nm] = nc.dram_tensor(nm, [P, SP // 16], mybir.dt.int16,
                                   kind="ExternalInput")
    out_d = nc.dram_tensor("out", [P, NB * FW], mybir.dt.float32,
                           kind="ExternalOutput")

    RG = [list(range(C))]

    with tile.TileContext(nc) as tc:
        with (
            tc.tile_pool(name="const", bufs=1) as cpool,
            tc.tile_pool(name="dram", bufs=1, space="DRAM") as dpool,
            tc.tile_pool(name="dramsh", bufs=2, space="DRAM") as shpool,
            tc.tile_pool(name="zp", bufs=3) as zpool,
            tc.tile_pool(name="lp", bufs=4) as lpool,
            tc.tile_pool(name="stgp", bufs=2) as stgpool,
            tc.tile_pool(name="pp", bufs=6, space="PSUM") as ppool,
            tc.tile_pool(name="ep", bufs=1) as epool,
        ):
            nc.gpsimd.load_library(mlp)

            ld4_sb = cpool.tile([P, VPT], mybir.dt.bfloat16)
            idxg_sb = cpool.tile([P, n_ch * CW], mybir.dt.int16)
            vals_sb = cpool.tile([P, T], mybir.dt.bfloat16)
            bnd_sb = {}
            for nm in ("bel", "bsl", "beh", "bsh"):
                bnd_sb[nm] = cpool.tile([P, SP // 16], mybir.dt.int16, name=nm)
                nc.sync.dma_start(out=bnd_sb[nm][:], in_=bnd_d[nm][:])
            s01_sb = cpool.tile([P, NB * FW], mybir.dt.float32)
            hn_bf = cpool.tile([P, NB * FW], mybir.dt.bfloat16)

            nc.sync.dma_start(out=ld4_sb[:], in_=ld4_d[:])
            nc.sync.dma_start(out=idxg_sb[:], in_=idxg_d[:])
            nc.sync.dma_start(out=vals_sb[:], in_=vals_d[:])

            nc.vector.memset(s01_sb[:], 0.0)
            nc.vector.memset(hn_bf[:], 0.0)

            TH = T - TL
            stage_l = dpool.tile([VPT, (TL + 1) * FW], mybir.dt.float32)
            stage_h = dpool.tile([VPT, (TH + 1) * FW], mybir.dt.float32)
            zblk = epool.tile([VPT, FW], mybir.dt.float32, name="zblk")
            nc.vector.memset(zblk[:], 0.0)
            nc.sync.dma_start(out=stage_l[:, TL * FW:(TL + 1) * FW], in_=zblk[:])
            nc.sync.dma_start(out=stage_h[:, TH * FW:(TH + 1) * FW], in_=zblk[:])

            # --- support = x @ W -> s01 (0.1x), h0 (bf16)
            s01v = s01_sb[:].rearrange("p (c f) -> p c f", f=FW)[:, :, :F]
            hnv = hn_bf[:].rearrange("p (c f) -> p c f", f=FW)[:, :, :F]
            with (
                tc.tile_pool(name="xtp", bufs=3) as xtp,
                tc.tile_pool(name="spp", bufs=2, space="PSUM") as spp,
                tc.tile_pool(name="scp", bufs=1) as scp,
            ):
                wp_sb = scp.tile([P, KC * F], mybir.dt.float32, name="wp_sb")
                nc.sync.dma_start(out=wp_sb[:], in_=wp_d[:])
                for m in range(NB):
                    xm = xtp.tile([P, KC * P], mybir.dt.float32, name="xm", tag="xm")
                    for k in range(KC):
                        nc.sync.dma_start(
                            out=xm[:, k * P:(k + 1) * P],
                            in_=xt_d[k * P:(k + 1) * P, m * P:(m + 1) * P])
                    ps = spp.tile([P, F], mybir.dt.float32, name="ps_sup",
                                  tag="ps_sup")
                    for k in range(KC):
                        nc.tensor.matmul(
                            out=ps[:], lhsT=xm[:, k * P:(k + 1) * P],
                            rhs=wp_sb[:, k * F:(k + 1) * F],
                            start=(k == 0), stop=(k == KC - 1))
                    dstv = s01v[:, m, :]
                    if m % 2 == 0:
                        nc.vector.tensor_copy(out=dstv, in_=ps[:])
                    else:
                        nc.scalar.copy(out=dstv, in_=ps[:])
                nc.vector.tensor_copy(out=hnv, in_=s01v)   # h0 = support (bf16)
                nc.vector.tensor_scalar_mul(s01v, s01v, ALPHA)

            # --- iterations
            for it in range(ITERS):
                agin = shpool.tile([SP, FW], mybir.dt.bfloat16, name="agin",
                                   tag="agin", bufs=2)
                hfull = shpool.tile([C * SP, FW], mybir.dt.bfloat16, name="hfull",
                                    tag="hfull", bufs=2, addr_space="Shared")
                nc.sync.dma_start(out=agin[:], in_=hn_bf[:])
                nc.gpsimd.collective_compute(
                    "AllGather", mybir.AluOpType.bypass, replica_groups=RG,
                    ins=[agin.opt()], outs=[hfull.opt()])

                l4b = ld4_sb[:].rearrange("p (o v) -> p o v", o=1) \
                    .to_broadcast([P, CH_TILES, VPT])
                for ci in range(n_ch):
                    in_lo = ci < TL // CH_TILES
                    tab = hfull[0:HALF, :] if in_lo else hfull[HALF:2 * HALF, :]
                    stg_t = stage_l if in_lo else stage_h
                    c0_loc = ci * CH_TILES - (0 if in_lo else TL)
                    z = zpool.tile([P, CH_TILES, FW], mybir.dt.bfloat16,
                                   name="z", tag="z")
                    nc.gpsimd.dma_gather(
                        out_ap=z[:], in_ap=tab,
                        idxs_ap=idxg_sb[:, ci * CW:(ci + 1) * CW],
                        num_idxs=CH_TILES * P, num_idxs_reg=CH_TILES * P,
                        elem_size=FW, single_packet=False)
                    # fold 0.9*val into the stationary operand (per-tile weights)
                    l4v = lpool.tile([P, CH_TILES, VPT], mybir.dt.bfloat16,
                                     name="l4v", tag="l4v")
                    vb = vals_sb[:, ci * CH_TILES:(ci + 1) * CH_TILES].rearrange(
                        "p (c o) -> p c o", o=1).to_broadcast([P, CH_TILES, VPT])
                    nc.vector.tensor_tensor(out=l4v[:], in0=l4b, in1=vb,
                                            op=mybir.AluOpType.mult)
                    for h2 in range(2):
                        HT = CH_TILES // 2
                        sg = stgpool.tile([VPT, HT * FW], mybir.dt.float32,
                                          name="sg", tag="sg")
                        for g in range(HT // MM_TILES):
                            pt = ppool.tile([VPT, MM_TILES * FW], mybir.dt.float32,
                                            name="pt", tag="pt")
                            for t in range(MM_TILES):
                                tt = h2 * HT + g * MM_TILES + t
                                nc.tensor.matmul(
                                    out=pt[:, t * FW:(t + 1) * FW],
                                    lhsT=l4v[:, tt, :], rhs=z[:, tt, :],
                                    start=True, stop=True)
                            if g % 2 == 0:
                                nc.vector.tensor_copy(
                                    out=sg[:, g * MM_TILES * FW:(g + 1) * MM_TILES * FW],
                                    in_=pt[:])
                            else:
                                nc.scalar.copy(
                                    out=sg[:, g * MM_TILES * FW:(g + 1) * MM_TILES * FW],
                                    in_=pt[:])
                        nc.sync.dma_start(
                            out=stg_t[:, (c0_loc + h2 * HT) * FW:
                                      (c0_loc + (h2 + 1) * HT) * FW],
                            in_=sg[:])

                # boundary gathers + combine, split into two node-halves so
                # half 1's gathers overlap half 0's arithmetic
                stgl_flat = stage_l[:].rearrange("v (t f) -> (v t) f", f=FW)
                stgh_flat = stage_h[:].rearrange("v (t f) -> (v t) f", f=FW)
                d = epool.tile([P, NB * FW], mybir.dt.float32, name="d", tag="d")
                NB0 = NB // 2 + 1
                for b0, b1 in ((0, NB0), (NB0, NB)):
                    nb_h = b1 - b0
                    if nb_h <= 0:
                        continue
                    gbt = {}
                    for nm in ("bel", "bsl", "beh", "bsh"):
                        gbt[nm] = epool.tile([P, nb_h, FW], mybir.dt.float32,
                                             name=f"g{nm}", tag=f"g{nm}")
                        flat = stgl_flat if nm.endswith("l") else stgh_flat
                        nc.gpsimd.dma_gather(
                            out_ap=gbt[nm][:], in_ap=flat,
                            idxs_ap=bnd_sb[nm][:, b0 * 8:b1 * 8],
                            num_idxs=nb_h * P, num_idxs_reg=nb_h * P,
                            elem_size=FW, single_packet=False)
                    fl = {k: v[:].rearrange("p c f -> p (c f)")
                          for k, v in gbt.items()}
                    dv = d[:, b0 * FW:b1 * FW]
                    nc.vector.tensor_tensor(out=dv, in0=fl["bel"], in1=fl["bsl"],
                                            op=mybir.AluOpType.subtract)
                    nc.vector.tensor_tensor(out=dv, in0=dv, in1=fl["beh"],
                                            op=mybir.AluOpType.add)
                    nc.vector.tensor_tensor(out=dv, in0=dv, in1=fl["bsh"],
                                            op=mybir.AluOpType.subtract)
                    nc.vector.tensor_tensor(out=dv, in0=dv,
                                            in1=s01_sb[:, b0 * FW:b1 * FW],
                                            op=mybir.AluOpType.add)
                    if it < ITERS - 1:
                        nc.scalar.activation(
                            out=hn_bf[:, b0 * FW:b1 * FW], in_=dv,
                            func=mybir.ActivationFunctionType.Relu)
                    else:
                        nc.scalar.activation(
                            out=dv, in_=dv,
                            func=mybir.ActivationFunctionType.Relu)
                if it == ITERS - 1:
                    nc.sync.dma_start(out=out_d[:], in_=d[:])

    nc.compile()
    return nc


# ----------------------------------------------------------------------------
# Entry point
# ----------------------------------------------------------------------------

_CACHE = {}


def _run(inputs, cfg, profile=False):
    from concourse.bass_utils import run_bass_kernel_spmd

    in_maps, meta = _prep(inputs, cfg)
    key = (cfg["N"], cfg["E"], meta["T"], meta["TL"])
    if key not in _CACHE:
        _CACHE[key] = _build(cfg, meta["T"], meta["TL"], meta["NB"])
    nc = _CACHE[key]
    res = run_bass_kernel_spmd(nc, in_maps, core_ids=list(range(cfg["CORES"])),
                               trace=profile)
    outs = []
    NB, SHARD = meta["NB"], meta["SHARD"]
    for c in range(cfg["CORES"]):
        arr = np.asarray(res.results[c]["out"])          # [128, NB*FW]
        arr = arr.reshape(P, NB, FW)[:, :, :OUT_F]
        arr = arr.transpose(1, 0, 2).reshape(NB * P, OUT_F)
        outs.append(arr[:SHARD])
    full = np.concatenate(outs, axis=0).astype(np.float32)
    return (full, res) if profile else full


def kernel(**inputs) -> np.ndarray:
    return _run(inputs, _DEF)
